# revision 29
# baseline (speedup 1.0000x reference)
"""Trainium2 Bass kernel for nn_CrossAttention_15006615733765 (raw Bass, no Tile).

Mathematical structure: the reference broadcasts a per-batch context vector
(B, CTX_DIM) to every spatial position before projecting to K/V.  All keys
within a batch are therefore identical, softmax over the key axis is exactly
uniform, and the attention output equals V itself.  The module collapses to

    out[b, c, h, w] = ((context[b] @ Wv) @ Wo + bo)[c]

independent of x, Wq and Wk (exact in infinite precision).  The kernel
computes the collapsed result on the tensor engine and materializes the
broadcast output shard per core, sharding the 512 output channels across the
8 cores (64 each).

All on-device data is fp16: the 2e-2 rel-err budget dwarfs fp16 rounding
(~5e-4 measured), the Wv stream halves to 768 KB, matmuls avoid the 4x fp32
LOW_HIGH penalty, and the output store halves to 1.18 MB (the host unshard
upcasts to fp32).

Dataflow (per core, all PE matmuls accumulate with start=False):
  stage 1  py1T[m][p, b] += sum_k Wv[k-chunk, m-chunk].T @ ctx[k-chunk]
           (column-major Wv slabs; y1 produced pre-transposed, one PSUM
           bank per column chunk m)
  copies   y1T[m] -> SBUF fp16 as each chunk finishes (overlaps the load)
  prep     prep[p, b, :] += y1T[:, m, b] (stride-0 broadcast lhsT)
                            @ Wo[m-chunk, cols]        [16 MMs, overlapped]
           prep[p, (b c)] += ones-row.T @ bias-tile    [1 MM, N=256]
  rep      one DVE copy replicates the fp32 prep row into NDUP fp16 copies
  store    two HWDGE DMAs (sync/scalar halves), 1.5 KiB descriptors

Correctness notes learned on HW:
  * start=True (first_mm) clears the whole 2 KiB PSUM bank, racing any
    sibling accumulation group in the same bank.  All matmuls here use
    start=False; the DVE zeroes the result banks at body start instead
    (overwrite-on-clear / accumulate-on-set is then correct regardless of
    stale has_written state).
  * Concurrent PE and DVE PSUM access (write/write to different banks, or
    read/write to the same bank) is fatal; warmups gate on the zeroing and
    each y1T chunk gets a private PSUM bank so its DVE copy never touches
    a bank the PE is still writing.
  * A dma_start always fires 16 semaphore increments, one per SDMA engine;
    transfers with <16 descriptors can fire padding increments before the
    data lands, so every gated load spans all 128 partitions (bias/ones
    rows ride inside the one cw tensor).

Performance notes (from per-instruction NTFF traces):
  * exec time ~= (store issue time) + ~9 us: the framework NEFF wrapper
    ends with a per-engine reset of ~200 semaphores after the exit
    rendezvous, which fully hides the output store; the only lever is
    reaching the store issue earlier.
  * Loads run ~150 GB/s per HWDGE queue (HBM->SBUF); the four Wv column
    slabs are spread over sync, scalar and the gpsimd SWDGE queue, consts
    first on sync (the scalar queue's first byte lands later than sync's).
"""

import numpy as np

import concourse.bacc as bacc
import concourse.mybir as mybir
from concourse.bass_utils import run_bass_kernel_spmd

B, DIM, CTX_DIM = 4, 512, 768
H = W = 48
NPOS = H * W
NCORES = 8
CPC = DIM // NCORES  # 64 output channels per core
P = 128
KC = CTX_DIM // P  # 6 contraction chunks for stage 1
KD = DIM // P      # 4 column chunks
ROW = B * CPC      # 256: one output row (all batches) per position
NDUP = 3           # replicated rows per partition -> 1.5 KiB descriptors
NREP = NPOS // (NDUP * P)  # 6 descriptor groups
F32 = mybir.dt.float32
F16 = mybir.dt.float16

# column offsets inside the packed consts tensor cw [P, CWN]
CTX0 = 0                  # ctx chunks: cw[p, CTX0 + k*B + b] = context[b, k*128+p]
WO0 = CTX0 + KC * B       # Wo slice:   cw[p, WO0 + m*CPC + c] = Wo[m*128+p, cols_i]
BT0 = WO0 + KD * CPC      # bias tile:  cw[0, BT0 + b*CPC + c] = bo[c] (partition 0)
ONE0 = BT0 + ROW          # ones row:   cw[0, ONE0 + j] = 1.0 (128 cols)
CWN = ONE0 + P

# slab placement: sync carries cw then m0, scalar m2 then m3, gpsimd m1.
# consume in expected arrival order, latest last.
M_ORDER = (2, 0, 1, 3)
M_GATE = {0: ("s", 32), 1: ("g", 16), 2: ("c", 16), 3: ("c", 32)}

_CACHE: dict = {}


def _build_nc():
    nc = bacc.Bacc("TRN2", target_bir_lowering=False, debug=False, num_devices=NCORES)

    # wvc[p, m*KC + k, c] = Wv[k*128+p, m*128+c]  (column-major slabs)
    wvc = nc.dram_tensor("wvc", [P, KD * KC, P], F16, kind="ExternalInput")
    cwc = nc.dram_tensor("cwc", [P, CWN], F16, kind="ExternalInput")
    outd = nc.dram_tensor("outd", [NPOS, ROW], F16, kind="ExternalOutput")

    wv_sb = nc.alloc_sbuf_tensor("wv_sb", [P, KD * KC, P], F16).ap()
    cw_sb = nc.alloc_sbuf_tensor("cw_sb", [P, CWN], F16).ap()
    y1T_sb = nc.alloc_sbuf_tensor("y1T_sb", [P, KD, B], F16).ap()
    rep_sb = nc.alloc_sbuf_tensor("rep_sb", [P, NDUP, ROW], F16).ap()
    warm_sb = nc.alloc_sbuf_tensor("warm_sb", [P, 512], F16).ap()

    py1T = [nc.alloc_psum_tensor(f"py1T{m}", [P, B], F32).ap() for m in range(KD)]
    prep = nc.alloc_psum_tensor("prep", [P, B, CPC], F32).ap()
    pwarm = nc.alloc_psum_tensor("pwarm", [P, 512], F32).ap()

    from contextlib import ExitStack

    with ExitStack() as stack:
        s_ws = stack.enter_context(nc.semaphore("s_ws"))
        s_wc = stack.enter_context(nc.semaphore("s_wc"))
        s_wg = stack.enter_context(nc.semaphore("s_wg"))
        s_pz = stack.enter_context(nc.semaphore("s_pz"))
        s_mm = stack.enter_context(nc.semaphore("s_mm"))
        s_y1 = stack.enter_context(nc.semaphore("s_y1"))
        s_pp = stack.enter_context(nc.semaphore("s_pp"))
        s_rep = stack.enter_context(nc.semaphore("s_rep"))
        s_out = stack.enter_context(nc.semaphore("s_out"))

        HREP = NREP // 2

        with nc.Block() as block:

            @block.sync
            def _(sync):
                sync.dma_start(out=cw_sb[:], in_=cwc[:]).then_inc(s_ws, 16)
                sync.dma_start(
                    out=wv_sb[:, 0:KC, :], in_=wvc[:, 0:KC, :]
                ).then_inc(s_ws, 16)
                sync.wait_ge(s_rep, 1)
                out_view = outd.rearrange("(r p d) n -> p r (d n)", p=P, d=NDUP)
                src_view = (
                    rep_sb.rearrange("p d n -> p (d n)")[:, None, :]
                    .broadcast_to((P, HREP, NDUP * ROW))
                )
                # No completion wait: the block-exit DRAIN on the issuing
                # engines waits for the HWDGE queues, so the semaphore-reset
                # epilogue overlaps the transfer.
                sync.dma_start(
                    out=out_view[:, :HREP, :], in_=src_view
                ).then_inc(s_out, 16)

            @block.scalar
            def _(scalar):
                for m in (2, 3):
                    scalar.dma_start(
                        out=wv_sb[:, m * KC:(m + 1) * KC, :],
                        in_=wvc[:, m * KC:(m + 1) * KC, :],
                    ).then_inc(s_wc, 16)
                scalar.wait_ge(s_rep, 1)
                out_view = outd.rearrange("(r p d) n -> p r (d n)", p=P, d=NDUP)
                src_view = (
                    rep_sb.rearrange("p d n -> p (d n)")[:, None, :]
                    .broadcast_to((P, HREP, NDUP * ROW))
                )
                scalar.dma_start(
                    out=out_view[:, HREP:, :], in_=src_view
                ).then_inc(s_out, 16)

            @block.gpsimd
            def _(g):
                g.dma_start(
                    out=wv_sb[:, 1 * KC:2 * KC, :], in_=wvc[:, 1 * KC:2 * KC, :]
                ).then_inc(s_wg, 16)

            @block.tensor
            def _(tensor):
                tensor.wait_ge(s_pz, 1)
                # HAM warmup: dummy matmuls on scratch SBUF keep the PE busy
                # so the 1.2->2.4 GHz unthrottle fires while stage 1 is
                # still load-gated (gated on s_pz: concurrent PE/DVE PSUM
                # writes are fatal).
                NWARM = 3
                for _w in range(NWARM):
                    nc.tensor.matmul(
                        pwarm[:],
                        warm_sb[:, 0:128],
                        warm_sb[:],
                        start=(_w == 0),
                        stop=(_w == NWARM - 1),
                    )

                # bias into prep: prep[p, (b c)] += 1 * bias-tile
                tensor.wait_ge(s_ws, 16)
                nc.tensor.matmul(
                    prep[:].rearrange("p b c -> p (b c)"),
                    cw_sb[0:1, ONE0:ONE0 + P],
                    cw_sb[0:1, BT0:BT0 + ROW],
                    start=False,
                    stop=False,
                    skip_group_check=True,
                )

                def stage1(m):
                    eng, val = M_GATE[m]
                    sem = {"s": s_ws, "c": s_wc, "g": s_wg}[eng]
                    tensor.wait_ge(sem, val)
                    ins = None
                    for k in range(KC):
                        ins = nc.tensor.matmul(
                            py1T[m][:],
                            wv_sb[:, m * KC + k, :],
                            cw_sb[:, CTX0 + k * B:CTX0 + (k + 1) * B],
                            start=False,
                            stop=(k == KC - 1),
                            skip_group_check=True,
                        )
                    ins.then_inc(s_mm, 1)

                def prep_mms(i, m, last=False):
                    # prep[p, b, :] += y1T[:, m, b] (bcast) @ Wo[m-chunk]
                    tensor.wait_ge(s_y1, i + 1)
                    ins = None
                    for b in range(B):
                        ins = nc.tensor.matmul(
                            prep[:, b, :],
                            y1T_sb[:, m, b:b + 1].broadcast_to((P, P)),
                            cw_sb[:, WO0 + m * CPC:WO0 + (m + 1) * CPC],
                            start=False,
                            stop=last,
                            skip_group_check=True,
                        )
                    if last:
                        ins.then_inc(s_pp, 1)

                stage1(M_ORDER[0])
                stage1(M_ORDER[1])
                prep_mms(0, M_ORDER[0])
                stage1(M_ORDER[2])
                prep_mms(1, M_ORDER[1])
                stage1(M_ORDER[3])
                prep_mms(2, M_ORDER[2])
                prep_mms(3, M_ORDER[3], last=True)

            @block.vector
            def _(vector):
                # Zero the PSUM result banks so the PE matmuls never need
                # start=True (whose whole-bank clear races sibling groups).
                for m in range(KD):
                    nc.vector.memset(py1T[m][:], 0.0)
                nc.vector.memset(prep[:], 0.0).then_inc(s_pz, 1)
                for i, m in enumerate(M_ORDER):
                    vector.wait_ge(s_mm, i + 1)
                    nc.vector.tensor_copy(
                        y1T_sb[:, m, :], py1T[m][:]
                    ).then_inc(s_y1, 1)
                vector.wait_ge(s_pp, 1)
                # single fused copy: fp32 prep row -> NDUP fp16 replicas
                flat = prep[:].rearrange("p b c -> p (b c)")
                nc.vector.tensor_copy(
                    rep_sb[:],
                    flat[:, None, :].broadcast_to((P, NDUP, ROW)),
                ).then_inc(s_rep, 1)

    nc.compile()
    return nc


def _get_nc():
    if "nc" not in _CACHE:
        _CACHE["nc"] = _build_nc()
    return _CACHE["nc"]


def _prepare_in_maps(context, Wv, Wo, bo):
    context = np.asarray(context, dtype=np.float32)
    Wv = np.asarray(Wv, dtype=np.float32)
    Wo = np.asarray(Wo, dtype=np.float32)
    bo = np.asarray(bo, dtype=np.float32)

    # wvc[p, m*KC+k, c] = Wv[k*128+p, m*128+c]  (column-major slabs)
    wvc = np.ascontiguousarray(
        Wv.astype(np.float16).reshape(KC, P, KD, P).transpose(1, 2, 0, 3)
        .reshape(P, KD * KC, P)
    )
    # ctx chunks: cw[p, CTX0 + k*B + b] = context[b, k*128+p]
    ctxc = (
        context.astype(np.float16).T.reshape(KC, P, B).transpose(1, 0, 2)
        .reshape(P, KC * B)
    )
    wo16 = Wo.astype(np.float16)
    bo16 = bo.astype(np.float16)

    in_maps = []
    for i in range(NCORES):
        cw = np.zeros((P, CWN), dtype=np.float16)
        cw[:, CTX0:CTX0 + KC * B] = ctxc
        # Wo slice: cw[p, WO0 + m*CPC + c] = Wo[m*128+p, i*CPC+c]
        cw[:, WO0:WO0 + KD * CPC] = (
            wo16[:, i * CPC:(i + 1) * CPC].reshape(KD, P, CPC)
            .transpose(1, 0, 2).reshape(P, KD * CPC)
        )
        cw[0, BT0:BT0 + ROW] = np.tile(bo16[i * CPC:(i + 1) * CPC], B)
        cw[0, ONE0:ONE0 + P] = 1.0
        in_maps.append({"wvc": wvc, "cwc": cw})
    return in_maps


def _unshard(results):
    shards = np.stack([r["outd"] for r in results], axis=0)
    shards = shards.reshape(NCORES, NPOS, B, CPC)
    out = shards.transpose(2, 0, 3, 1).reshape(B, DIM, H, W)
    return np.ascontiguousarray(out.astype(np.float32))


def kernel(x, context, Wq, Wk, Wv, Wo, bo):
    del x, Wq, Wk
    nc = _get_nc()
    in_maps = _prepare_in_maps(context, Wv, Wo, bo)
    results = run_bass_kernel_spmd(nc, in_maps, list(range(NCORES))).results
    return _unshard(results)


# revision 32
# speedup vs baseline: 1.0572x; 1.0572x over previous
"""Trainium2 Bass kernel for nn_CrossAttention_15006615733765 (raw Bass, no Tile).

Mathematical structure: the reference broadcasts a per-batch context vector
(B, CTX_DIM) to every spatial position before projecting to K/V.  All keys
within a batch are therefore identical, softmax over the key axis is exactly
uniform, and the attention output equals V itself.  The module collapses to

    out[b, c, h, w] = ((context[b] @ Wv) @ Wo + bo)[c]

independent of x, Wq and Wk (exact in infinite precision).  The kernel
computes the collapsed result on the tensor engine and materializes the
broadcast output shard per core, sharding the 512 output channels across the
8 cores (64 each).

All on-device data is fp16: the 2e-2 rel-err budget dwarfs fp16 rounding
(~5e-4 measured), the Wv stream halves to 768 KB, matmuls avoid the 4x fp32
LOW_HIGH penalty, and the output store halves to 1.18 MB (the host unshard
upcasts to fp32).

Dataflow (per core, all PE matmuls accumulate with start=False):
  stage 1  py1T[m][p, b] += sum_k Wv[k-chunk, m-chunk].T @ ctx[k-chunk]
           (column-major Wv slabs; y1 produced pre-transposed, one PSUM
           bank per column chunk m)
  copies   y1T[m] -> SBUF fp16 as each chunk finishes (overlaps the load)
  prep     prep[p, b, :] += y1T[:, m, b] (stride-0 broadcast lhsT)
                            @ Wo[m-chunk, cols]        [16 MMs, overlapped]
           prep[p, (b c)] += ones-row.T @ bias-tile    [1 MM, N=256]
  rep      one DVE copy replicates the fp32 prep row into NDUP fp16 copies
  store    two HWDGE DMAs (sync/scalar halves), 1.5 KiB descriptors

Correctness notes learned on HW:
  * start=True (first_mm) clears the whole 2 KiB PSUM bank, racing any
    sibling accumulation group in the same bank.  All matmuls here use
    start=False; the DVE zeroes the result banks at body start instead
    (overwrite-on-clear / accumulate-on-set is then correct regardless of
    stale has_written state).
  * Concurrent PE and DVE PSUM access (write/write to different banks, or
    read/write to the same bank) is fatal; warmups gate on the zeroing and
    each y1T chunk gets a private PSUM bank so its DVE copy never touches
    a bank the PE is still writing.
  * A dma_start always fires 16 semaphore increments, one per SDMA engine;
    transfers with <16 descriptors can fire padding increments before the
    data lands, so every gated load spans all 128 partitions (bias/ones
    rows ride inside the one cw tensor).

Performance notes (from per-instruction NTFF traces):
  * exec time ~= (store issue time) + ~9 us: the framework NEFF wrapper
    ends with a per-engine reset of ~200 semaphores after the exit
    rendezvous, which fully hides the output store; the only lever is
    reaching the store issue earlier.
  * Loads run ~150 GB/s per HWDGE queue (HBM->SBUF); the four Wv column
    slabs are spread over sync, scalar and the gpsimd SWDGE queue, consts
    first on sync (the scalar queue's first byte lands later than sync's).
"""

import numpy as np

import concourse.bacc as bacc
import concourse.mybir as mybir
from concourse.bass_utils import run_bass_kernel_spmd

B, DIM, CTX_DIM = 4, 512, 768
H = W = 48
NPOS = H * W
NCORES = 8
CPC = DIM // NCORES  # 64 output channels per core
P = 128
KC = CTX_DIM // P  # 6 contraction chunks for stage 1
KD = DIM // P      # 4 column chunks
ROW = B * CPC      # 256: one output row (all batches) per position
NDUP = 3           # replicated rows per partition -> 1.5 KiB descriptors
NREP = NPOS // (NDUP * P)  # 6 descriptor groups
F32 = mybir.dt.float32
F16 = mybir.dt.float16

# column offsets inside the packed consts tensor cw [P, CWN]
CTX0 = 0                  # ctx chunks: cw[p, CTX0 + k*B + b] = context[b, k*128+p]
WO0 = CTX0 + KC * B       # Wo slice:   cw[p, WO0 + m*CPC + c] = Wo[m*128+p, cols_i]
BO0 = WO0 + KD * CPC      # bias row:  cw[0, BO0 + c] = bo[c] (partition 0)
IC0 = BO0 + CPC           # indicator: cw[r, IC0] = (r == 0)
CWN = IC0 + 1

# slab placement: sync carries cw, bias, m0, m1; scalar m2, m3.
# consume in expected arrival order, latest last.  The 1-descriptor bias
# tensor is issued between cw and m0, so m0's full-width completion
# (s_ws>=32) implies it landed (per-engine FIFO rings).
M_ORDER = (2, 0, 3, 1)
M_GATE = {0: ("s", 32), 1: ("s", 48), 2: ("c", 16), 3: ("c", 32)}

_CACHE: dict = {}


def _build_nc():
    nc = bacc.Bacc("TRN2", target_bir_lowering=False, debug=False, num_devices=NCORES)

    # wvc[p, m*KC + k, c] = Wv[k*128+p, m*128+c]  (column-major slabs)
    wvc = nc.dram_tensor("wvc", [P, KD * KC, P], F16, kind="ExternalInput")
    cwc = nc.dram_tensor("cwc", [P, CWN], F16, kind="ExternalInput")
    outd = nc.dram_tensor("outd", [NPOS, ROW], F16, kind="ExternalOutput")

    wv_sb = nc.alloc_sbuf_tensor("wv_sb", [P, KD * KC, P], F16).ap()
    cw_sb = nc.alloc_sbuf_tensor("cw_sb", [P, CWN], F16).ap()
    y1T_sb = nc.alloc_sbuf_tensor("y1T_sb", [P, KD, B], F16).ap()
    rep_sb = nc.alloc_sbuf_tensor("rep_sb", [P, NDUP, ROW], F16).ap()
    warm_sb = nc.alloc_sbuf_tensor("warm_sb", [P, 512], F16).ap()

    py1T = [nc.alloc_psum_tensor(f"py1T{m}", [P, B], F32).ap() for m in range(KD)]
    prep = nc.alloc_psum_tensor("prep", [P, B, CPC], F32).ap()
    pwarm = nc.alloc_psum_tensor("pwarm", [P, 512], F32).ap()

    from contextlib import ExitStack

    with ExitStack() as stack:
        s_ws = stack.enter_context(nc.semaphore("s_ws"))
        s_wc = stack.enter_context(nc.semaphore("s_wc"))
        s_pz = stack.enter_context(nc.semaphore("s_pz"))
        s_mm = stack.enter_context(nc.semaphore("s_mm"))
        s_y1 = stack.enter_context(nc.semaphore("s_y1"))
        s_pp = stack.enter_context(nc.semaphore("s_pp"))
        s_rep = stack.enter_context(nc.semaphore("s_rep"))
        s_out = stack.enter_context(nc.semaphore("s_out"))

        HREP = NREP // 2

        with nc.Block() as block:

            @block.sync
            def _(sync):
                sync.dma_start(out=cw_sb[:], in_=cwc[:]).then_inc(s_ws, 16)
                for m in (0, 1):
                    sync.dma_start(
                        out=wv_sb[:, m * KC:(m + 1) * KC, :],
                        in_=wvc[:, m * KC:(m + 1) * KC, :],
                    ).then_inc(s_ws, 16)
                sync.wait_ge(s_rep, 1)
                out_view = outd.rearrange("(r p d) n -> p r (d n)", p=P, d=NDUP)
                src_view = (
                    rep_sb.rearrange("p d n -> p (d n)")[:, None, :]
                    .broadcast_to((P, HREP, NDUP * ROW))
                )
                # No completion wait: the block-exit DRAIN on the issuing
                # engines waits for the HWDGE queues, so the semaphore-reset
                # epilogue overlaps the transfer.
                sync.dma_start(
                    out=out_view[:, :HREP, :], in_=src_view
                ).then_inc(s_out, 16)

            @block.scalar
            def _(scalar):
                for m in (2, 3):
                    scalar.dma_start(
                        out=wv_sb[:, m * KC:(m + 1) * KC, :],
                        in_=wvc[:, m * KC:(m + 1) * KC, :],
                    ).then_inc(s_wc, 16)
                scalar.wait_ge(s_rep, 1)
                out_view = outd.rearrange("(r p d) n -> p r (d n)", p=P, d=NDUP)
                src_view = (
                    rep_sb.rearrange("p d n -> p (d n)")[:, None, :]
                    .broadcast_to((P, HREP, NDUP * ROW))
                )
                scalar.dma_start(
                    out=out_view[:, HREP:, :], in_=src_view
                ).then_inc(s_out, 16)

            @block.tensor
            def _(tensor):
                tensor.wait_ge(s_pz, 1)
                # HAM warmup: dummy matmuls on scratch SBUF keep the PE busy
                # so the 1.2->2.4 GHz unthrottle fires while stage 1 is
                # still load-gated (gated on s_pz: concurrent PE/DVE PSUM
                # writes are fatal).
                NWARM = 3
                for _w in range(NWARM):
                    nc.tensor.matmul(
                        pwarm[:],
                        warm_sb[:, 0:128],
                        warm_sb[:],
                        start=(_w == 0),
                        stop=(_w == NWARM - 1),
                    )

                # bias into prep: prep[p, b, :] += indicator.T @ bias row
                tensor.wait_ge(s_ws, 16)
                for b in range(B):
                    nc.tensor.matmul(
                        prep[:, b, :],
                        cw_sb[0:B, IC0:IC0 + 1].broadcast_to((B, P)),
                        cw_sb[0:B, BO0:BO0 + CPC],
                        start=False,
                        stop=False,
                        skip_group_check=True,
                    )

                def stage1(m):
                    eng, val = M_GATE[m]
                    sem = {"s": s_ws, "c": s_wc}[eng]
                    tensor.wait_ge(sem, val)
                    ins = None
                    for k in range(KC):
                        ins = nc.tensor.matmul(
                            py1T[m][:],
                            wv_sb[:, m * KC + k, :],
                            cw_sb[:, CTX0 + k * B:CTX0 + (k + 1) * B],
                            start=False,
                            stop=(k == KC - 1),
                            skip_group_check=True,
                        )
                    ins.then_inc(s_mm, 1)

                def prep_mms(i, m, last=False):
                    # prep[p, b, :] += y1T[:, m, b] (bcast) @ Wo[m-chunk]
                    tensor.wait_ge(s_y1, i + 1)
                    ins = None
                    for b in range(B):
                        ins = nc.tensor.matmul(
                            prep[:, b, :],
                            y1T_sb[:, m, b:b + 1].broadcast_to((P, P)),
                            cw_sb[:, WO0 + m * CPC:WO0 + (m + 1) * CPC],
                            start=False,
                            stop=last,
                            skip_group_check=True,
                        )
                    if last:
                        ins.then_inc(s_pp, 1)

                stage1(M_ORDER[0])
                stage1(M_ORDER[1])
                prep_mms(0, M_ORDER[0])
                stage1(M_ORDER[2])
                prep_mms(1, M_ORDER[1])
                stage1(M_ORDER[3])
                prep_mms(2, M_ORDER[2])
                prep_mms(3, M_ORDER[3], last=True)

            @block.vector
            def _(vector):
                # Zero the PSUM result banks so the PE matmuls never need
                # start=True (whose whole-bank clear races sibling groups).
                for m in range(KD):
                    nc.vector.memset(py1T[m][:], 0.0)
                nc.vector.memset(prep[:], 0.0).then_inc(s_pz, 1)
                for i, m in enumerate(M_ORDER):
                    vector.wait_ge(s_mm, i + 1)
                    nc.vector.tensor_copy(
                        y1T_sb[:, m, :], py1T[m][:]
                    ).then_inc(s_y1, 1)
                vector.wait_ge(s_pp, 1)
                flat = prep[:].rearrange("p b c -> p (b c)")
                nc.vector.tensor_copy(rep_sb[:, 0, :], flat)
                # replicas 1..NDUP-1 in one SBUF->SBUF copy (broadcast source)
                nc.vector.tensor_copy(
                    rep_sb[:, 1:, :],
                    rep_sb[:, 0:1, :].broadcast_to((P, NDUP - 1, ROW)),
                ).then_inc(s_rep, 1)

    nc.compile()
    return nc


def _get_nc():
    if "nc" not in _CACHE:
        _CACHE["nc"] = _build_nc()
    return _CACHE["nc"]


def _prepare_in_maps(context, Wv, Wo, bo):
    context = np.asarray(context, dtype=np.float32)
    Wv = np.asarray(Wv, dtype=np.float32)
    Wo = np.asarray(Wo, dtype=np.float32)
    bo = np.asarray(bo, dtype=np.float32)

    # wvc[p, m*KC+k, c] = Wv[k*128+p, m*128+c]  (column-major slabs)
    wvc = np.ascontiguousarray(
        Wv.astype(np.float16).reshape(KC, P, KD, P).transpose(1, 2, 0, 3)
        .reshape(P, KD * KC, P)
    )
    # ctx chunks: cw[p, CTX0 + k*B + b] = context[b, k*128+p]
    ctxc = (
        context.astype(np.float16).T.reshape(KC, P, B).transpose(1, 0, 2)
        .reshape(P, KC * B)
    )
    wo16 = Wo.astype(np.float16)
    bo16 = bo.astype(np.float16)

    in_maps = []
    for i in range(NCORES):
        cw = np.zeros((P, CWN), dtype=np.float16)
        cw[:, CTX0:CTX0 + KC * B] = ctxc
        # Wo slice: cw[p, WO0 + m*CPC + c] = Wo[m*128+p, i*CPC+c]
        cw[:, WO0:WO0 + KD * CPC] = (
            wo16[:, i * CPC:(i + 1) * CPC].reshape(KD, P, CPC)
            .transpose(1, 0, 2).reshape(P, KD * CPC)
        )
        cw[0, BO0:BO0 + CPC] = bo16[i * CPC:(i + 1) * CPC]
        cw[0, IC0] = 1.0
        in_maps.append({"wvc": wvc, "cwc": cw})
    return in_maps


def _unshard(results):
    shards = np.stack([r["outd"] for r in results], axis=0)
    shards = shards.reshape(NCORES, NPOS, B, CPC)
    out = shards.transpose(2, 0, 3, 1).reshape(B, DIM, H, W)
    return np.ascontiguousarray(out.astype(np.float32))


def kernel(x, context, Wq, Wk, Wv, Wo, bo):
    del x, Wq, Wk
    nc = _get_nc()
    in_maps = _prepare_in_maps(context, Wv, Wo, bo)
    results = run_bass_kernel_spmd(nc, in_maps, list(range(NCORES))).results
    return _unshard(results)


# revision 33
# speedup vs baseline: 1.0685x; 1.0107x over previous
"""Trainium2 Bass kernel for nn_CrossAttention_15006615733765 (raw Bass, no Tile).

Mathematical structure: the reference broadcasts a per-batch context vector
(B, CTX_DIM) to every spatial position before projecting to K/V.  All keys
within a batch are therefore identical, softmax over the key axis is exactly
uniform, and the attention output equals V itself.  The module collapses to

    out[b, c, h, w] = ((context[b] @ Wv) @ Wo + bo)[c]

independent of x, Wq and Wk (exact in infinite precision).  The kernel
computes the collapsed result on the tensor engine and materializes the
broadcast output shard per core, sharding the 512 output channels across the
8 cores (64 each).

All on-device data is fp16: the 2e-2 rel-err budget dwarfs fp16 rounding
(~5e-4 measured), the Wv stream halves to 768 KB, matmuls avoid the 4x fp32
LOW_HIGH penalty, and the output store halves to 1.18 MB (the host unshard
upcasts to fp32).

Dataflow (per core, all PE matmuls accumulate with start=False):
  stage 1  py1T[m][p, b] += sum_k Wv[k-chunk, m-chunk].T @ ctx[k-chunk]
           (column-major Wv slabs; y1 produced pre-transposed, one PSUM
           bank per column chunk m)
  copies   y1T[m] -> SBUF fp16 as each chunk finishes (overlaps the load)
  prep     prep[p, b, :] += y1T[:, m, b] (stride-0 broadcast lhsT)
                            @ Wo[m-chunk, cols]        [16 MMs, overlapped]
           prep[p, (b c)] += ones-row.T @ bias-tile    [1 MM, N=256]
  rep      one DVE copy replicates the fp32 prep row into NDUP fp16 copies
  store    two HWDGE DMAs (sync/scalar halves), 1.5 KiB descriptors

Correctness notes learned on HW:
  * start=True (first_mm) clears the whole 2 KiB PSUM bank, racing any
    sibling accumulation group in the same bank.  All matmuls here use
    start=False; the DVE zeroes the result banks at body start instead
    (overwrite-on-clear / accumulate-on-set is then correct regardless of
    stale has_written state).
  * Concurrent PE and DVE PSUM access (write/write to different banks, or
    read/write to the same bank) is fatal; warmups gate on the zeroing and
    each y1T chunk gets a private PSUM bank so its DVE copy never touches
    a bank the PE is still writing.
  * A dma_start always fires 16 semaphore increments, one per SDMA engine;
    transfers with <16 descriptors can fire padding increments before the
    data lands, so every gated load spans all 128 partitions (bias/ones
    rows ride inside the one cw tensor).

Performance notes (from per-instruction NTFF traces):
  * exec time ~= (store issue time) + ~9 us: the framework NEFF wrapper
    ends with a per-engine reset of ~200 semaphores after the exit
    rendezvous, which fully hides the output store; the only lever is
    reaching the store issue earlier.
  * Loads run ~150 GB/s per HWDGE queue (HBM->SBUF); the four Wv column
    slabs are spread over sync, scalar and the gpsimd SWDGE queue, consts
    first on sync (the scalar queue's first byte lands later than sync's).
"""

import numpy as np

import concourse.bacc as bacc
import concourse.mybir as mybir
from concourse.bass_utils import run_bass_kernel_spmd

B, DIM, CTX_DIM = 4, 512, 768
H = W = 48
NPOS = H * W
NCORES = 8
CPC = DIM // NCORES  # 64 output channels per core
P = 128
KC = CTX_DIM // P  # 6 contraction chunks for stage 1
KD = DIM // P      # 4 column chunks
ROW = B * CPC      # 256: one output row (all batches) per position
NDUP = 3           # replicated rows per partition -> 1.5 KiB descriptors
NREP = NPOS // (NDUP * P)  # 6 descriptor groups
F32 = mybir.dt.float32
F16 = mybir.dt.float16

# column offsets inside the packed consts tensor cw [P, CWN]
CTX0 = 0                  # ctx chunks: cw[p, CTX0 + k*B + b] = context[b, k*128+p]
WO0 = CTX0 + KC * B       # Wo slice:   cw[p, WO0 + m*CPC + c] = Wo[m*128+p, cols_i]
BO0 = WO0 + KD * CPC      # bias row:  cw[0, BO0 + c] = bo[c] (partition 0)
IC0 = BO0 + CPC           # indicator: cw[r, IC0] = (r == 0)
CWN = IC0 + 1

# slab placement: sync carries cw, bias, m0, m1; scalar m2, m3.
# consume in expected arrival order, latest last.  The 1-descriptor bias
# tensor is issued between cw and m0, so m0's full-width completion
# (s_ws>=32) implies it landed (per-engine FIFO rings).
M_ORDER = (2, 0, 3, 1)
M_GATE = {0: ("s", 32), 1: ("s", 48), 2: ("c", 16), 3: ("c", 32)}

_CACHE: dict = {}


def _build_nc():
    nc = bacc.Bacc("TRN2", target_bir_lowering=False, debug=False, num_devices=NCORES)

    # wvc[m, p, k*128+c] = Wv[k*128+p, m*128+c]: each column slab m is a
    # contiguous 192 KB DRAM block so its descriptors are DRAM-adjacent
    # across partitions (HBM locality; strided slabs measured ~145 GB/s
    # per queue vs ~255 for contiguous).
    wvc = nc.dram_tensor("wvc", [KD, P, KC * P], F16, kind="ExternalInput")
    cwc = nc.dram_tensor("cwc", [P, CWN], F16, kind="ExternalInput")
    outd = nc.dram_tensor("outd", [NPOS, ROW], F16, kind="ExternalOutput")

    wv_sb = nc.alloc_sbuf_tensor("wv_sb", [P, KD * KC, P], F16).ap()
    cw_sb = nc.alloc_sbuf_tensor("cw_sb", [P, CWN], F16).ap()
    y1T_sb = nc.alloc_sbuf_tensor("y1T_sb", [P, KD, B], F16).ap()
    rep_sb = nc.alloc_sbuf_tensor("rep_sb", [P, NDUP, ROW], F16).ap()
    warm_sb = nc.alloc_sbuf_tensor("warm_sb", [P, 512], F16).ap()

    py1T = [nc.alloc_psum_tensor(f"py1T{m}", [P, B], F32).ap() for m in range(KD)]
    prep = nc.alloc_psum_tensor("prep", [P, B, CPC], F32).ap()
    pwarm = nc.alloc_psum_tensor("pwarm", [P, 512], F32).ap()

    from contextlib import ExitStack

    with ExitStack() as stack:
        s_ws = stack.enter_context(nc.semaphore("s_ws"))
        s_wc = stack.enter_context(nc.semaphore("s_wc"))
        s_pz = stack.enter_context(nc.semaphore("s_pz"))
        s_mm = stack.enter_context(nc.semaphore("s_mm"))
        s_y1 = stack.enter_context(nc.semaphore("s_y1"))
        s_pp = stack.enter_context(nc.semaphore("s_pp"))
        s_rep = stack.enter_context(nc.semaphore("s_rep"))
        s_out = stack.enter_context(nc.semaphore("s_out"))

        HREP = NREP // 2

        with nc.Block() as block:

            @block.sync
            def _(sync):
                sync.dma_start(out=cw_sb[:], in_=cwc[:]).then_inc(s_ws, 16)
                for m in (0, 1):
                    sync.dma_start(
                        out=wv_sb[:, m * KC:(m + 1) * KC, :],
                        in_=wvc[m, :, :],
                    ).then_inc(s_ws, 16)
                sync.wait_ge(s_rep, 1)
                out_view = outd.rearrange("(r p d) n -> p r (d n)", p=P, d=NDUP)
                src_view = (
                    rep_sb.rearrange("p d n -> p (d n)")[:, None, :]
                    .broadcast_to((P, HREP, NDUP * ROW))
                )
                # No completion wait: the block-exit DRAIN on the issuing
                # engines waits for the HWDGE queues, so the semaphore-reset
                # epilogue overlaps the transfer.
                sync.dma_start(
                    out=out_view[:, :HREP, :], in_=src_view
                ).then_inc(s_out, 16)

            @block.scalar
            def _(scalar):
                for m in (2, 3):
                    scalar.dma_start(
                        out=wv_sb[:, m * KC:(m + 1) * KC, :],
                        in_=wvc[m, :, :],
                    ).then_inc(s_wc, 16)
                scalar.wait_ge(s_rep, 1)
                out_view = outd.rearrange("(r p d) n -> p r (d n)", p=P, d=NDUP)
                src_view = (
                    rep_sb.rearrange("p d n -> p (d n)")[:, None, :]
                    .broadcast_to((P, HREP, NDUP * ROW))
                )
                scalar.dma_start(
                    out=out_view[:, HREP:, :], in_=src_view
                ).then_inc(s_out, 16)

            @block.tensor
            def _(tensor):
                tensor.wait_ge(s_pz, 1)
                # HAM warmup: dummy matmuls on scratch SBUF keep the PE busy
                # so the 1.2->2.4 GHz unthrottle fires while stage 1 is
                # still load-gated (gated on s_pz: concurrent PE/DVE PSUM
                # writes are fatal).
                NWARM = 3
                for _w in range(NWARM):
                    nc.tensor.matmul(
                        pwarm[:],
                        warm_sb[:, 0:128],
                        warm_sb[:],
                        start=(_w == 0),
                        stop=(_w == NWARM - 1),
                    )

                # bias into prep: prep[p, b, :] += indicator.T @ bias row
                tensor.wait_ge(s_ws, 16)
                for b in range(B):
                    nc.tensor.matmul(
                        prep[:, b, :],
                        cw_sb[0:B, IC0:IC0 + 1].broadcast_to((B, P)),
                        cw_sb[0:B, BO0:BO0 + CPC],
                        start=False,
                        stop=False,
                        skip_group_check=True,
                    )

                def stage1(m):
                    eng, val = M_GATE[m]
                    sem = {"s": s_ws, "c": s_wc}[eng]
                    tensor.wait_ge(sem, val)
                    ins = None
                    for k in range(KC):
                        ins = nc.tensor.matmul(
                            py1T[m][:],
                            wv_sb[:, m * KC + k, :],
                            cw_sb[:, CTX0 + k * B:CTX0 + (k + 1) * B],
                            start=False,
                            stop=(k == KC - 1),
                            skip_group_check=True,
                        )
                    ins.then_inc(s_mm, 1)

                def prep_mms(i, m, last=False):
                    # prep[p, b, :] += y1T[:, m, b] (bcast) @ Wo[m-chunk]
                    tensor.wait_ge(s_y1, i + 1)
                    ins = None
                    for b in range(B):
                        ins = nc.tensor.matmul(
                            prep[:, b, :],
                            y1T_sb[:, m, b:b + 1].broadcast_to((P, P)),
                            cw_sb[:, WO0 + m * CPC:WO0 + (m + 1) * CPC],
                            start=False,
                            stop=last,
                            skip_group_check=True,
                        )
                    if last:
                        ins.then_inc(s_pp, 1)

                stage1(M_ORDER[0])
                stage1(M_ORDER[1])
                prep_mms(0, M_ORDER[0])
                stage1(M_ORDER[2])
                prep_mms(1, M_ORDER[1])
                stage1(M_ORDER[3])
                prep_mms(2, M_ORDER[2])
                prep_mms(3, M_ORDER[3], last=True)

            @block.vector
            def _(vector):
                # Zero the PSUM result banks so the PE matmuls never need
                # start=True (whose whole-bank clear races sibling groups).
                for m in range(KD):
                    nc.vector.memset(py1T[m][:], 0.0)
                nc.vector.memset(prep[:], 0.0).then_inc(s_pz, 1)
                for i, m in enumerate(M_ORDER):
                    vector.wait_ge(s_mm, i + 1)
                    nc.vector.tensor_copy(
                        y1T_sb[:, m, :], py1T[m][:]
                    ).then_inc(s_y1, 1)
                vector.wait_ge(s_pp, 1)
                flat = prep[:].rearrange("p b c -> p (b c)")
                nc.vector.tensor_copy(rep_sb[:, 0, :], flat)
                # replicas 1..NDUP-1 in one SBUF->SBUF copy (broadcast source)
                nc.vector.tensor_copy(
                    rep_sb[:, 1:, :],
                    rep_sb[:, 0:1, :].broadcast_to((P, NDUP - 1, ROW)),
                ).then_inc(s_rep, 1)

    nc.compile()
    return nc


def _get_nc():
    if "nc" not in _CACHE:
        _CACHE["nc"] = _build_nc()
    return _CACHE["nc"]


def _prepare_in_maps(context, Wv, Wo, bo):
    context = np.asarray(context, dtype=np.float32)
    Wv = np.asarray(Wv, dtype=np.float32)
    Wo = np.asarray(Wo, dtype=np.float32)
    bo = np.asarray(bo, dtype=np.float32)

    # wvc[m, p, k*128+c] = Wv[k*128+p, m*128+c]  (contiguous column slabs)
    wvc = np.ascontiguousarray(
        Wv.astype(np.float16).reshape(KC, P, KD, P).transpose(2, 1, 0, 3)
        .reshape(KD, P, KC * P)
    )
    # ctx chunks: cw[p, CTX0 + k*B + b] = context[b, k*128+p]
    ctxc = (
        context.astype(np.float16).T.reshape(KC, P, B).transpose(1, 0, 2)
        .reshape(P, KC * B)
    )
    wo16 = Wo.astype(np.float16)
    bo16 = bo.astype(np.float16)

    in_maps = []
    for i in range(NCORES):
        cw = np.zeros((P, CWN), dtype=np.float16)
        cw[:, CTX0:CTX0 + KC * B] = ctxc
        # Wo slice: cw[p, WO0 + m*CPC + c] = Wo[m*128+p, i*CPC+c]
        cw[:, WO0:WO0 + KD * CPC] = (
            wo16[:, i * CPC:(i + 1) * CPC].reshape(KD, P, CPC)
            .transpose(1, 0, 2).reshape(P, KD * CPC)
        )
        cw[0, BO0:BO0 + CPC] = bo16[i * CPC:(i + 1) * CPC]
        cw[0, IC0] = 1.0
        in_maps.append({"wvc": wvc, "cwc": cw})
    return in_maps


def _unshard(results):
    shards = np.stack([r["outd"] for r in results], axis=0)
    shards = shards.reshape(NCORES, NPOS, B, CPC)
    out = shards.transpose(2, 0, 3, 1).reshape(B, DIM, H, W)
    return np.ascontiguousarray(out.astype(np.float32))


def kernel(x, context, Wq, Wk, Wv, Wo, bo):
    del x, Wq, Wk
    nc = _get_nc()
    in_maps = _prepare_in_maps(context, Wv, Wo, bo)
    results = run_bass_kernel_spmd(nc, in_maps, list(range(NCORES))).results
    return _unshard(results)


# revision 34
# speedup vs baseline: 1.0710x; 1.0023x over previous
"""Trainium2 Bass kernel for nn_CrossAttention_15006615733765 (raw Bass, no Tile).

Mathematical structure: the reference broadcasts a per-batch context vector
(B, CTX_DIM) to every spatial position before projecting to K/V.  All keys
within a batch are therefore identical, softmax over the key axis is exactly
uniform, and the attention output equals V itself.  The module collapses to

    out[b, c, h, w] = ((context[b] @ Wv) @ Wo + bo)[c]

independent of x, Wq and Wk (exact in infinite precision).  The kernel
computes the collapsed result on the tensor engine and materializes the
broadcast output shard per core, sharding the 512 output channels across the
8 cores (64 each).

All on-device data is fp16: the 2e-2 rel-err budget dwarfs fp16 rounding
(~5e-4 measured), the Wv stream halves to 768 KB, matmuls avoid the 4x fp32
LOW_HIGH penalty, and the output store halves to 1.18 MB (the host unshard
upcasts to fp32).

Dataflow (per core, all PE matmuls accumulate with start=False):
  loads    ctx+bias first on sync, then four Wv column slabs split over
           the two HWDGE queues; each slab carries its own Wo chunk so all
           downstream work for a slab gates only on that slab.
  stage 1  py1T[m][p, b] += sum_k Wv[k-chunk, m-chunk].T @ ctx[k-chunk]
           (y1 produced pre-transposed, one PSUM bank per column chunk)
  copies   y1T[m] -> SBUF fp16 as each chunk finishes (overlaps the load)
  prep     prep[p, b, :] += y1T[:, m, b] (stride-0 broadcast lhsT)
                            @ Wo[m-chunk, cols]       [16 MMs, overlapped]
           prep[p, b, :] += indicator.T @ bias row    [4 MMs, overlapped]
  rep      one DVE cast copy prep -> fp16 row (no replication: NDUP=1)
  store    two HWDGE DMAs (sync/scalar halves), 512 B descriptors; the
           ~8.5 us framework epilogue hides the store drain completely.

Correctness notes learned on HW:
  * start=True (first_mm) clears the whole 2 KiB PSUM bank, racing any
    sibling accumulation group in the same bank.  All matmuls here use
    start=False; the DVE zeroes the result banks at body start instead
    (overwrite-on-clear / accumulate-on-set is then correct regardless of
    stale has_written state).
  * Concurrent PE and DVE PSUM access (write/write to different banks, or
    read/write to the same bank) is fatal; warmups gate on the zeroing and
    each y1T chunk gets a private PSUM bank so its DVE copy never touches
    a bank the PE is still writing.
  * A dma_start always fires 16 semaphore increments, one per SDMA engine;
    transfers with <16 descriptors can fire padding increments before the
    data lands, so every gated load spans all 128 partitions.
  * Single-partition DRAM tensors break walrus dynamic-DMA codegen (and
    can wedge the device); bias/indicator columns ride the 128-partition
    ctx tensor instead.

Performance notes (from per-instruction NTFF traces):
  * exec time ~= (store issue time) + ~8.8 us: the framework NEFF wrapper
    ends with a per-engine reset of ~200 semaphores after the exit
    rendezvous, which fully hides the output store; the only lever is
    reaching the store issue earlier.
  * HBM->SBUF loads cap at ~150 GB/s per HWDGE queue (descriptor-latency
    bound; DRAM contiguity and descriptor size barely move it), so the
    load is split evenly across both queues and all consumers are gated
    at column-slab granularity to overlap everything else with the load.
"""

import numpy as np

import concourse.bacc as bacc
import concourse.mybir as mybir
from concourse.bass_utils import run_bass_kernel_spmd

B, DIM, CTX_DIM = 4, 512, 768
H = W = 48
NPOS = H * W
NCORES = 8
CPC = DIM // NCORES  # 64 output channels per core
P = 128
KC = CTX_DIM // P  # 6 contraction chunks for stage 1
KD = DIM // P      # 4 column chunks
ROW = B * CPC      # 256: one output row (all batches) per position
F32 = mybir.dt.float32
F16 = mybir.dt.float16

# ctx+bias tensor cb [P, CBN]
CTX0 = 0                  # ctx chunks: cb[p, CTX0 + k*B + b] = context[b, k*128+p]
BO0 = CTX0 + KC * B       # bias row:   cb[0, BO0 + c] = bo[c] (partition 0)
IC0 = BO0 + CPC           # indicator:  cb[r, IC0] = (r == 0)
CBN = IC0 + 1

# per-slab tensor wvc [KD, P, SLN]: Wv column chunk + its Wo chunk
WVN = KC * P              # cols 0:768   Wv[k*128+p, m*128+c] at k*128+c
SLO = WVN                 # cols 768:832 Wo[m*128+p, i*64+c]
SLN = SLO + CPC

# slab placement: sync carries cb, m0, m1; scalar m2, m3.
M_ORDER = (0, 2, 1, 3)
M_GATE = {0: ("s", 32), 1: ("s", 48), 2: ("c", 16), 3: ("c", 32)}

_CACHE: dict = {}


def _build_nc():
    nc = bacc.Bacc("TRN2", target_bir_lowering=False, debug=False, num_devices=NCORES)

    wvc = nc.dram_tensor("wvc", [KD, P, SLN], F16, kind="ExternalInput")
    cbc = nc.dram_tensor("cbc", [P, CBN], F16, kind="ExternalInput")
    outd = nc.dram_tensor("outd", [NPOS, ROW], F16, kind="ExternalOutput")

    wv_sb = nc.alloc_sbuf_tensor("wv_sb", [P, KD, SLN], F16).ap()
    cb_sb = nc.alloc_sbuf_tensor("cb_sb", [P, CBN], F16).ap()
    y1T_sb = nc.alloc_sbuf_tensor("y1T_sb", [P, KD, B], F16).ap()
    rep_sb = nc.alloc_sbuf_tensor("rep_sb", [P, ROW], F16).ap()
    warm_sb = nc.alloc_sbuf_tensor("warm_sb", [P, 512], F16).ap()

    py1T = [nc.alloc_psum_tensor(f"py1T{m}", [P, B], F32).ap() for m in range(KD)]
    prep = nc.alloc_psum_tensor("prep", [P, B, CPC], F32).ap()
    pwarm = nc.alloc_psum_tensor("pwarm", [P, 512], F32).ap()

    from contextlib import ExitStack

    with ExitStack() as stack:
        s_ws = stack.enter_context(nc.semaphore("s_ws"))
        s_wc = stack.enter_context(nc.semaphore("s_wc"))
        s_pz = stack.enter_context(nc.semaphore("s_pz"))
        s_mm = stack.enter_context(nc.semaphore("s_mm"))
        s_y1 = stack.enter_context(nc.semaphore("s_y1"))
        s_pp = stack.enter_context(nc.semaphore("s_pp"))
        s_rep = stack.enter_context(nc.semaphore("s_rep"))
        s_out = stack.enter_context(nc.semaphore("s_out"))

        # pos = r*P + p: each partition writes NREP1 512 B rows
        NREP1 = NPOS // P  # 18
        HREP = NREP1 // 2

        with nc.Block() as block:

            @block.sync
            def _(sync):
                sync.dma_start(out=cb_sb[:], in_=cbc[:]).then_inc(s_ws, 16)
                for m in (0, 1):
                    sync.dma_start(
                        out=wv_sb[:, m, :], in_=wvc[m, :, :]
                    ).then_inc(s_ws, 16)
                sync.wait_ge(s_rep, 1)
                out_view = outd.rearrange("(r p) n -> p r n", p=P)
                src_view = rep_sb[:, None, :].broadcast_to((P, HREP, ROW))
                # No completion wait: the block-exit DRAIN on the issuing
                # engines waits for the HWDGE queues, so the semaphore-reset
                # epilogue overlaps the transfer.
                sync.dma_start(
                    out=out_view[:, :HREP, :], in_=src_view
                ).then_inc(s_out, 16)

            @block.scalar
            def _(scalar):
                for m in (2, 3):
                    scalar.dma_start(
                        out=wv_sb[:, m, :], in_=wvc[m, :, :]
                    ).then_inc(s_wc, 16)
                scalar.wait_ge(s_rep, 1)
                out_view = outd.rearrange("(r p) n -> p r n", p=P)
                src_view = rep_sb[:, None, :].broadcast_to((P, HREP, ROW))
                scalar.dma_start(
                    out=out_view[:, HREP:, :], in_=src_view
                ).then_inc(s_out, 16)

            @block.tensor
            def _(tensor):
                tensor.wait_ge(s_pz, 1)
                # HAM warmup: dummy matmuls on scratch SBUF keep the PE busy
                # so the 1.2->2.4 GHz unthrottle fires while stage 1 is
                # still load-gated (gated on s_pz: concurrent PE/DVE PSUM
                # writes are fatal).
                NWARM = 3
                for _w in range(NWARM):
                    nc.tensor.matmul(
                        pwarm[:],
                        warm_sb[:, 0:128],
                        warm_sb[:],
                        start=(_w == 0),
                        stop=(_w == NWARM - 1),
                    )

                # bias into prep: prep[p, b, :] += indicator.T @ bias row
                tensor.wait_ge(s_ws, 16)
                for b in range(B):
                    nc.tensor.matmul(
                        prep[:, b, :],
                        cb_sb[0:B, IC0:IC0 + 1].broadcast_to((B, P)),
                        cb_sb[0:B, BO0:BO0 + CPC],
                        start=False,
                        stop=False,
                        skip_group_check=True,
                    )

                def stage1(m):
                    eng, val = M_GATE[m]
                    tensor.wait_ge(s_ws if eng == "s" else s_wc, val)
                    ins = None
                    for k in range(KC):
                        ins = nc.tensor.matmul(
                            py1T[m][:],
                            wv_sb[:, m, k * P:(k + 1) * P],
                            cb_sb[:, CTX0 + k * B:CTX0 + (k + 1) * B],
                            start=False,
                            stop=(k == KC - 1),
                            skip_group_check=True,
                        )
                    ins.then_inc(s_mm, 1)

                def prep_mms(i, m, last=False):
                    # prep[p, b, :] += y1T[:, m, b] (bcast) @ Wo[m-chunk]
                    tensor.wait_ge(s_y1, i + 1)
                    ins = None
                    for b in range(B):
                        ins = nc.tensor.matmul(
                            prep[:, b, :],
                            y1T_sb[:, m, b:b + 1].broadcast_to((P, P)),
                            wv_sb[:, m, SLO:SLO + CPC],
                            start=False,
                            stop=last,
                            skip_group_check=True,
                        )
                    if last:
                        ins.then_inc(s_pp, 1)

                stage1(M_ORDER[0])
                stage1(M_ORDER[1])
                prep_mms(0, M_ORDER[0])
                stage1(M_ORDER[2])
                prep_mms(1, M_ORDER[1])
                stage1(M_ORDER[3])
                prep_mms(2, M_ORDER[2])
                prep_mms(3, M_ORDER[3], last=True)

            @block.vector
            def _(vector):
                # Zero the PSUM result banks so the PE matmuls never need
                # start=True (whose whole-bank clear races sibling groups).
                for m in range(KD):
                    nc.vector.memset(py1T[m][:], 0.0)
                nc.vector.memset(prep[:], 0.0).then_inc(s_pz, 1)
                for i, m in enumerate(M_ORDER):
                    vector.wait_ge(s_mm, i + 1)
                    nc.vector.tensor_copy(
                        y1T_sb[:, m, :], py1T[m][:]
                    ).then_inc(s_y1, 1)
                vector.wait_ge(s_pp, 1)
                flat = prep[:].rearrange("p b c -> p (b c)")
                nc.vector.tensor_copy(rep_sb[:], flat).then_inc(s_rep, 1)

    nc.compile()
    return nc


def _get_nc():
    if "nc" not in _CACHE:
        _CACHE["nc"] = _build_nc()
    return _CACHE["nc"]


def _prepare_in_maps(context, Wv, Wo, bo):
    context = np.asarray(context, dtype=np.float32)
    Wv = np.asarray(Wv, dtype=np.float32)
    Wo = np.asarray(Wo, dtype=np.float32)
    bo = np.asarray(bo, dtype=np.float32)

    # wv part: wvc[m, p, k*128+c] = Wv[k*128+p, m*128+c]
    wvp = (
        Wv.astype(np.float16).reshape(KC, P, KD, P).transpose(2, 1, 0, 3)
        .reshape(KD, P, KC * P)
    )
    # ctx chunks: cb[p, CTX0 + k*B + b] = context[b, k*128+p]
    ctxc = (
        context.astype(np.float16).T.reshape(KC, P, B).transpose(1, 0, 2)
        .reshape(P, KC * B)
    )
    wo16 = Wo.astype(np.float16)
    bo16 = bo.astype(np.float16)

    in_maps = []
    for i in range(NCORES):
        wvc = np.empty((KD, P, SLN), dtype=np.float16)
        wvc[:, :, :WVN] = wvp
        # Wo chunk per slab: wvc[m, p, SLO + c] = Wo[m*128+p, i*CPC+c]
        wvc[:, :, SLO:] = (
            wo16[:, i * CPC:(i + 1) * CPC].reshape(KD, P, CPC)
        )
        cb = np.zeros((P, CBN), dtype=np.float16)
        cb[:, CTX0:CTX0 + KC * B] = ctxc
        cb[0, BO0:BO0 + CPC] = bo16[i * CPC:(i + 1) * CPC]
        cb[0, IC0] = 1.0
        in_maps.append({"wvc": wvc, "cbc": cb})
    return in_maps


def _unshard(results):
    shards = np.stack([r["outd"] for r in results], axis=0)
    shards = shards.reshape(NCORES, NPOS, B, CPC)
    out = shards.transpose(2, 0, 3, 1).reshape(B, DIM, H, W)
    return np.ascontiguousarray(out.astype(np.float32))


def kernel(x, context, Wq, Wk, Wv, Wo, bo):
    del x, Wq, Wk
    nc = _get_nc()
    in_maps = _prepare_in_maps(context, Wv, Wo, bo)
    results = run_bass_kernel_spmd(nc, in_maps, list(range(NCORES))).results
    return _unshard(results)
